# revision 1
# baseline (speedup 1.0000x reference)
"""DINOv2 self-attention (QKV projection + SDPA, no out-proj) on 8 Trainium2
NeuronCores.

Sharding: pure data-parallel over batch (B=8 -> one batch element per core);
no cross-core communication.

Host-side prep inside kernel(): transpose hidden_states to x.T per batch and
pack W as W.T = [Wq.T | Wk.T | Wv.T], so every on-chip matmul operand already
has its contraction dim on the partition axis.

Per-core program (S=1370, D=1024, H=16, hd=64), all matmuls in float32r
(TF32-like, full PE rate at moving-dim >= 256):
  1. qT = (x Wq^T + bq)^T, kT likewise: out.T = W.T^T @ x.T with W.T tiles
     stationary; bias added on DVE during PSUM->SBUF copy.  Layout [o, s]
     puts head_dim on partitions for attention.
  2. v in standard layout [s, o] (x.T tiles stationary, W.T moving), written
     interleaved with a ones-column per head: v_ext[:, t, h*65+64] = 1 so the
     softmax denominator falls out of the ctx matmul as an extra output row.
  3. Per head-pair (two heads share an o-tile at partition 0/64 -> concurrent
     PE row-groups), per sq-chunk (<=512): scoresT[sk, sq] = kT^T @ qT,
     exp via ACT with fused 1/8 scale (softmax max-subtraction skipped:
     |scores/8| <= ~6), ctx.T[65, sq] accumulated over the 11 sk tiles with
     [v | 1] stationary.
  4. ctx.T (+denominator row) transposed back with PE transpose per 128-wide
     sub-tile, then out[:, h*64:h*64+64] = ctx * (1/denom) on DVE.
"""

import numpy as np
from contextlib import ExitStack

import concourse.bass as bass
import concourse.bacc as bacc
import concourse.tile as tile
from concourse import mybir
from concourse import bass_utils
from concourse.masks import make_identity

S, D, H, HD = 1370, 1024, 16, 64
F32 = mybir.dt.float32
F32R = mybir.dt.float32r
ND = D // 128                      # 8 contraction tiles
NO = D // 128                      # 8 output tiles per projection
NT = (S + 127) // 128              # 11 token tiles
TSZ = [min(128, S - i * 128) for i in range(NT)]
CHUNKS = [(0, 512), (512, 512), (1024, S - 1024)]
EXP = mybir.ActivationFunctionType.Exp
CTX_LAG = 0   # how many kt iterations ctx matmuls lag behind scores
INTERLEAVE = False  # process two head-pairs' kt loops interleaved
PSC2 = False  # psc bufs=2 / pss bufs=2 instead of 1/3
_SKIP_ATTN = False


def _body(tc, xT, wT, bT, bvb, out, reps=1):
    nc = tc.nc
    with ExitStack() as ctx:
        const = ctx.enter_context(tc.tile_pool(name="const", bufs=1))
        ident = const.tile([65, 65], F32)
        make_identity(nc, ident)
        bT_sb = const.tile([128, 24], F32)
        nc.sync.dma_start(bT_sb[:], bT[:])

        qk_pool = ctx.enter_context(tc.tile_pool(name="qkT", bufs=1))
        vext_pool = ctx.enter_context(tc.tile_pool(name="vext", bufs=1))
        for _rep in range(reps):
            _one_pass(tc, ctx, qk_pool, vext_pool, ident, bT_sb, bvb,
                      xT, wT, out)


def _one_pass(tc, ctx, qk_pool, vext_pool, ident, bT_sb, bvb, xT, wT, out):
        nc = tc.nc
        qT = qk_pool.tile([128, NO, S], F32R, tag="qT", name="qT")
        kT = qk_pool.tile([128, NO, S], F32R, tag="kT", name="kT")
        v_ext = vext_pool.tile([128, NT, H * 65], F32R, tag="vext", name="v_ext")
        # ones columns (h*65+64) for the fused softmax denominator; memset
        # can't produce f32r, so synthesize 1.0 as in0*0 + 1 on DVE
        for t in range(NT):
            ones_view = v_ext[:, t, :].rearrange("p (h e) -> p h e", e=65)[:, :, 64]
            nc.vector.tensor_scalar(
                ones_view, bT_sb[:, 0:16],
                0.0, 1.0, mybir.AluOpType.mult, mybir.AluOpType.add)

        with ExitStack() as s1:
            xt_pool = s1.enter_context(tc.tile_pool(name="xt", bufs=1))
            xt = xt_pool.tile([128, ND, S], F32R)

            # ---- v = x @ Wv^T + bv, scattered into v_ext ----
            with ExitStack() as s2:
                wv_pool = s2.enter_context(tc.tile_pool(name="wv", bufs=1))
                bvb_sb = wv_pool.tile([128, D], F32, tag="bvb", name="bvb_sb")
                nc.scalar.dma_start(bvb_sb[:], bvb[:])
                psv = s2.enter_context(
                    tc.tile_pool(name="psv", bufs=4, space="PSUM"))
                for half in range(2):
                    wv = wv_pool.tile([128, ND, 512], F32R, tag="wv", name="wv")
                    for d in range(ND):
                        c = 2 * D + half * 512
                        if half == 0:
                            # interleave x and Wv loads across the two HWDGE
                            # queues so the first v matmul starts early
                            nc.sync.dma_start(
                                xt[:, d, :], xT[d * 128:(d + 1) * 128, :])
                        nc.scalar.dma_start(
                            wv[:, d, :], wT[d * 128:(d + 1) * 128, c:c + 512])
                    for t in range(NT):
                        tsz = TSZ[t]
                        ps = psv.tile([128, 512], F32, tag="psv", name="psv")
                        for d in range(ND):
                            nc.tensor.matmul(
                                ps[:tsz, :], xt[:, d, t * 128:t * 128 + tsz],
                                wv[:, d, :], start=(d == 0), stop=(d == ND - 1))
                        dst = v_ext[:tsz, t, :].rearrange(
                            "p (h e) -> p h e", e=65)[:, half * 8:(half + 1) * 8, 0:64]
                        src = ps[:tsz, :].rearrange("p (h e) -> p h e", e=64)
                        bias = bvb_sb[:tsz, half * 512:(half + 1) * 512].rearrange(
                            "p (h e) -> p h e", e=64)
                        nc.vector.tensor_add(dst, src, bias)

            # ---- qT / kT projections (W loaded 2 o-tiles per DMA) ----
            with ExitStack() as s3:
                wqk_pool = s3.enter_context(tc.tile_pool(name="wqk", bufs=9))
                psqk = s3.enter_context(
                    tc.tile_pool(name="psqk", bufs=4, space="PSUM"))
                for og in range(0, NO, 2):
                    for proj in (1, 0):
                        dstT = qT if proj == 0 else kT
                        ws = []
                        for d in range(ND):
                            w = wqk_pool.tile([128, 256], F32R,
                                              tag="wqk", name="wqk")
                            c = proj * D + og * 128
                            nc.scalar.dma_start(
                                w[:], wT[d * 128:(d + 1) * 128, c:c + 256])
                            ws.append(w)
                        for oo in range(2):
                            o = og + oo
                            # chunk-innermost: each stationary W tile serves
                            # 3 consecutive matmuls before switching
                            pss3 = [psqk.tile([128, 512], F32,
                                              tag="psqk", name="psqk")
                                    for _ in CHUNKS]
                            for d in range(ND):
                                for (ci, (c0, cw)) in enumerate(CHUNKS):
                                    nc.tensor.matmul(
                                        pss3[ci][:, :cw],
                                        ws[d][:, oo * 128:(oo + 1) * 128],
                                        xt[:, d, c0:c0 + cw],
                                        start=(d == 0), stop=(d == ND - 1))
                            for (ci, (c0, cw)) in enumerate(CHUNKS):
                                nc.vector.tensor_scalar_add(
                                    dstT[:, o, c0:c0 + cw], pss3[ci][:, :cw],
                                    bT_sb[:, proj * 8 + o:proj * 8 + o + 1])

        # ---- attention ----
        if _SKIP_ATTN:
            # diagnostic mode: write junk output straight from qT
            with ExitStack() as s4:
                outp = s4.enter_context(tc.tile_pool(name="outp", bufs=2))
                for t in range(NT):
                    ot = outp.tile([128, D], F32, tag="out", name="out_sb")
                    nc.vector.tensor_copy(
                        ot[:TSZ[t], :],
                        qT[:, 0, 0:D].bitcast(F32)[:TSZ[t], :])
                    nc.sync.dma_start(out[t * 128:t * 128 + TSZ[t], :],
                                      ot[:TSZ[t], :])
            return
        with ExitStack() as s4:
            pss = s4.enter_context(tc.tile_pool(
                name="pss", bufs=2 if (INTERLEAVE or PSC2) else 3, space="PSUM"))
            psc = s4.enter_context(tc.tile_pool(
                name="psc", bufs=2 if (INTERLEAVE or PSC2) else 1, space="PSUM"))
            tpp = pss
            et_pool = s4.enter_context(tc.tile_pool(name="et", bufs=3))
            cs_pool = s4.enter_context(tc.tile_pool(name="cs", bufs=17))
            outp = s4.enter_context(tc.tile_pool(name="outp", bufs=7))
            rec_pool = s4.enter_context(tc.tile_pool(name="rec", bufs=4))

            def flush(fcsts, fouts, fsub, fc0):
                # batched finalize for a completed chunk, emitted inside the
                # NEXT chunk's first head-pair window so the PE transposes
                # hide under ACT's exp stream
                for (h, cst) in fcsts:
                    for (si, (s0, ssz)) in enumerate(fsub):
                        tp = tpp.tile([128, 65], F32, tag="pss", name="tp")
                        nc.tensor.transpose(
                            tp[:ssz, :], cst[:, s0:s0 + ssz], ident[:65, :65])
                        rec = rec_pool.tile([128, 1], F32, tag="rec",
                                            name="rec")
                        nc.vector.reciprocal(rec[:ssz], tp[:ssz, 64:65])
                        nc.vector.tensor_scalar_mul(
                            fouts[si][:ssz, h * 64:(h + 1) * 64],
                            tp[:ssz, 0:64], rec[:ssz])
                for (si, (s0, ssz)) in enumerate(fsub):
                    nc.sync.dma_start(
                        out[fc0 + s0:fc0 + s0 + ssz, :], fouts[si][:ssz, :])

            pending = None
            for (c0, cw) in CHUNKS:
                sub = [(s0, min(128, cw - s0)) for s0 in range(0, cw, 128)]
                outs = []
                for _ in sub:
                    outs.append(outp.tile([128, D], F32, tag="out", name="out_sb"))

                def attn_pairs(hps):
                    # one or two head-pairs, kt loops interleaved
                    pcs = {hp: psc.tile([65, 2, 512], F32, tag="psc", name="psc")
                           for hp in hps}
                    ets = {}

                    def emit_ctx(hp, kt):
                        ksz = TSZ[kt]
                        for hi in range(2):
                            h = 2 * hp + hi
                            nc.tensor.matmul(
                                pcs[hp][:, hi, :cw],
                                v_ext[:ksz, kt, h * 65:(h + 1) * 65],
                                ets.pop((hp, kt))[:ksz, hi, :cw] if hi else
                                ets[(hp, kt)][:ksz, hi, :cw],
                                start=(kt == 0), stop=(kt == NT - 1))

                    for kt in range(NT):
                        k0, ksz = kt * 128, TSZ[kt]
                        for hp in hps:
                            ps_s = pss.tile([128, 2, 512], F32, tag="pss",
                                            name="pss")
                            et = et_pool.tile([128, 2, 512], F32R, tag="et",
                                              name="et")
                            ets[(hp, kt)] = et
                            for hi in range(2):
                                p0 = hi * 64
                                nc.tensor.matmul(
                                    ps_s[:ksz, hi, :cw],
                                    kT[p0:p0 + 64, hp, k0:k0 + ksz],
                                    qT[p0:p0 + 64, hp, c0:c0 + cw],
                                    start=True, stop=True)
                            nc.scalar.activation(
                                et[:ksz, :, :cw], ps_s[:ksz, :, :cw], EXP,
                                scale=0.125)
                            if kt >= CTX_LAG:
                                emit_ctx(hp, kt - CTX_LAG)
                    for r in range(NT - CTX_LAG, NT):
                        for hp in hps:
                            emit_ctx(hp, r)
                    for hp in hps:
                        for hi in range(2):
                            h = 2 * hp + hi
                            cst = cs_pool.tile([65, 512], F32, tag="cs",
                                               name="cs")
                            nc.vector.tensor_copy(cst[:, :cw], pcs[hp][:, hi, :cw])
                            csts.append((h, cst))

                csts = []
                for hp in range(8):
                    attn_pairs((hp,))
                    if hp == 0 and pending is not None:
                        flush(*pending)
                        pending = None
                pending = (csts, outs, sub, c0)
            flush(*pending)


def build_program(reps=1):
    nc = bacc.Bacc("TRN2", target_bir_lowering=False, debug=False,
                   num_devices=8)
    xT = nc.dram_tensor("xT", [D, S], F32R, kind="ExternalInput").ap()
    wT = nc.dram_tensor("wT", [D, 3 * D], F32R, kind="ExternalInput").ap()
    bT = nc.dram_tensor("bT", [128, 24], F32, kind="ExternalInput").ap()
    bvb = nc.dram_tensor("bvb", [128, D], F32, kind="ExternalInput").ap()
    out = nc.dram_tensor("out", [S, D], F32, kind="ExternalOutput").ap()
    with tile.TileContext(nc) as tc:
        _body(tc, xT, wT, bT, bvb, out, reps=reps)
    nc.compile()
    return nc


_PROGRAM = None


def _get_program():
    global _PROGRAM
    if _PROGRAM is None:
        _PROGRAM = build_program()
    return _PROGRAM


def _prep_inputs(hidden_states, Wq, bq, Wk, bk, Wv, bv):
    hs = np.asarray(hidden_states, dtype=np.float32)
    B = hs.shape[0]
    xT = np.ascontiguousarray(hs.transpose(0, 2, 1))
    wT = np.ascontiguousarray(np.concatenate(
        [np.asarray(Wq, dtype=np.float32).T,
         np.asarray(Wk, dtype=np.float32).T,
         np.asarray(Wv, dtype=np.float32).T], axis=1))
    b_all = np.concatenate([np.asarray(bq, dtype=np.float32),
                            np.asarray(bk, dtype=np.float32),
                            np.asarray(bv, dtype=np.float32)])
    bT_np = np.ascontiguousarray(b_all.reshape(24, 128).T)
    bvb_np = np.ascontiguousarray(
        np.broadcast_to(np.asarray(bv, dtype=np.float32), (128, D)))
    return [{"xT": xT[b], "wT": wT, "bT": bT_np, "bvb": bvb_np}
            for b in range(B)]


def run(in_maps, **kw):
    nc = _get_program()
    return bass_utils.run_bass_kernel_spmd(
        nc, in_maps, core_ids=list(range(len(in_maps))), **kw)


def kernel(hidden_states, Wq, bq, Wk, bk, Wv, bv):
    in_maps = _prep_inputs(hidden_states, Wq, bq, Wk, bk, Wv, bv)
    res = run(in_maps)
    return np.stack([res.results[b]["out"] for b in range(len(in_maps))],
                    axis=0)



# revision 18
# speedup vs baseline: 1.0134x; 1.0134x over previous
"""DINOv2 self-attention (QKV projection + SDPA, no out-proj) on 8 Trainium2
NeuronCores.

Sharding: pure data-parallel over batch (B=8 -> one batch element per core);
no cross-core communication.

v2 design (vs the f32r phase-serial baseline):
  * All matmul operands in bf16 (error vs f32 reference ~0.8% of out absmax,
    well inside the 2e-2 gate); PSUM accumulation stays f32.
  * Per head-pair (hp) pipeline: the QKV projection for hp+1 is emitted as
    "filler" PE instructions interleaved into attention(hp)'s score/ctx
    stream, so the PE never idles while ACT (exp) runs and ACT never idles
    waiting for a serial projection phase.
  * Swapped ctx matmul: stationary = exp(scores) subtile [sk,128sq] (SBUF
    bf16), moving = [v | 1] per head (65 cols) -> psum [sq, 65] accumulated
    over the 11 sk tiles.  Output lands directly in [token, feature] layout:
    no PE transposes, no psum->sbuf->psum round trip; col 64 is the softmax
    denominator (ones column trick).
  * exp on ACT with fused 1/8 scale, psum f32 in -> bf16 SBUF out; ctx lags
    scores by 2 kt so et(kt) is always ready when its ctx matmuls issue.

Engine budget per core/shot: PE ~280us (proj 110 + scores 100 + ctx 52),
ACT ~250us, DVE ~110us, DMA ~45us -> PE-bound ~300us once overlapped.
"""

import numpy as np
from collections import deque

import concourse.bass as bass
import concourse.bacc as bacc
import concourse.tile as tile
from concourse import mybir
from concourse import bass_utils
from concourse.bass import _add_dep_helper

S, D, H, HD = 1370, 1024, 16, 64
ND = D // 128                      # 8 contraction tiles
NT = (S + 127) // 128              # 11 token tiles
TSZ = [min(128, S - i * 128) for i in range(NT)]
CHUNKS = [(0, 512), (512, 512), (1024, S - 1024)]
NHP = 8                            # head pairs (= o-tiles per projection)
BF = mybir.dt.bfloat16
F32 = mybir.dt.float32
EXP = mybir.ActivationFunctionType.Exp
CTX_LAG = 2
FILLER_CYC = 1050                  # PE cycles of proj filler per (chunk,kt) slot


def _proj_fillers(nc, hp, x, w_all, qkT, v_ext, bqk_sb, bvb_sb, psproj):
    """Yield (cycles, closure) pairs emitting the projection for head-pair hp.

    Per closure: one PE matmul or one DVE drain. Accumulation groups use
    rotating psproj bufs so consecutive groups can overlap.
    """
    # v projection: per token tile, out [tsz, 128feat]; stationary x d-tile,
    # moving Wv slice.
    for t in range(NT):
        tsz = TSZ[t]
        ps = psproj.tile([128, 512], F32, tag="psproj", name="psproj")
        for d in range(ND):
            def mm(d=d, t=t, tsz=tsz, ps=ps):
                nc.tensor.matmul(
                    ps[:tsz, 0:128], x[:, d, t * 128:t * 128 + tsz],
                    w_all[:, d, 256:384], start=(d == 0), stop=(d == ND - 1))
            yield 150, mm

        def drain(t=t, tsz=tsz, ps=ps):
            dst = v_ext[:tsz, t, :, 0:64]
            src = ps[:tsz, 0:128].rearrange("p (i e) -> p i e", e=64)
            bias = bvb_sb[:tsz, hp * 128:(hp + 1) * 128].rearrange(
                "p (i e) -> p i e", e=64)
            nc.vector.tensor_add(dst, src, bias)
        yield 0, drain

    # q/k projections: per chunk, out [128feat, cw]; stationary W slice,
    # moving x chunk.
    for proj in (0, 1):
        for (c0, cw) in CHUNKS:
            ps = psproj.tile([128, 512], F32, tag="psproj", name="psproj")
            for d in range(ND):
                def mm(proj=proj, c0=c0, cw=cw, d=d, ps=ps):
                    nc.tensor.matmul(
                        ps[:, :cw], w_all[:, d, proj * 128:(proj + 1) * 128],
                        x[:, d, c0:c0 + cw], start=(d == 0), stop=(d == ND - 1))
                yield cw, mm

            def drain(proj=proj, c0=c0, cw=cw, ps=ps):
                nc.vector.tensor_scalar_add(
                    qkT[:, proj, c0:c0 + cw], ps[:, :cw],
                    bqk_sb[:, proj * 8 + hp:proj * 8 + hp + 1])
            yield 0, drain


def _body(tc, xT, wT, bqk, bvb, out, reps=1):
    nc = tc.nc
    with tc.tile_pool(name="const", bufs=1) as const, \
         tc.tile_pool(name="x", bufs=2) as x_pool, \
         tc.tile_pool(name="w", bufs=2) as w_pool, \
         tc.tile_pool(name="qk", bufs=2) as qk_pool, \
         tc.tile_pool(name="vext", bufs=2) as vext_pool, \
         tc.tile_pool(name="et", bufs=4) as et_pool, \
         tc.tile_pool(name="rec", bufs=2) as rec_pool, \
         tc.tile_pool(name="cs", bufs=2) as cs_pool, \
         tc.tile_pool(name="outp", bufs=2) as out_pool, \
         tc.tile_pool(name="pss", bufs=2, space="PSUM") as pss, \
         tc.tile_pool(name="pctx", bufs=1, space="PSUM") as pctx_pool, \
         tc.tile_pool(name="psproj", bufs=2, space="PSUM") as psproj:

        bqk_sb = const.tile([128, 16], F32)
        nc.scalar.dma_start(bqk_sb[:], bqk[:])
        bvb_sb = const.tile([128, D], F32)
        nc.scalar.dma_start(bvb_sb[:], bvb[:])
        zeros_sb = const.tile([128, 512], BF)
        nc.vector.memset(zeros_sb[:], 0.0)
        prev_cst = [None]

        def load_x():
            x = x_pool.tile([128, ND, S], BF, tag="x", name="x")
            for d in range(ND):
                nc.sync.dma_start(x[:, d, :], xT[d * 128:(d + 1) * 128, :])
            return x

        def load_w(hp):
            w_all = w_pool.tile([128, ND, 384], BF, tag="w", name="w")
            for d in range(ND):
                nc.scalar.dma_start(
                    w_all[:, d, :], wT[d * 128:(d + 1) * 128,
                                       hp * 384:(hp + 1) * 384])
            return w_all

        def proj_tiles(hp):
            qkT = qk_pool.tile([128, 2, S], BF, tag="qk", name="qkT")
            v_ext = vext_pool.tile([128, NT, 2, 65], BF, tag="vext",
                                   name="v_ext")
            nc.vector.memset(v_ext[:, :, :, 64:65], 1.0)
            return qkT, v_ext

        # ---- prologue: x + W(hp0) + straight-line proj(hp0) ----
        steps = [(r, hp) for r in range(reps) for hp in range(NHP)]
        x_cur = load_x()
        w_cur = load_w(0)
        qkT_cur, vext_cur = proj_tiles(0)
        for _, fn in _proj_fillers(nc, 0, x_cur, w_cur, qkT_cur, vext_cur,
                                   bqk_sb, bvb_sb, psproj):
            fn()

        for si, (r, hp) in enumerate(steps):
            # stage the next step's inputs + build its filler stream
            fillers = deque()
            if si + 1 < len(steps):
                r2, hp2 = steps[si + 1]
                x_nxt = load_x() if hp2 == 0 else x_cur
                w_nxt = load_w(hp2)
                qkT_nxt, vext_nxt = proj_tiles(hp2)
                fillers = deque(_proj_fillers(
                    nc, hp2, x_nxt, w_nxt, qkT_nxt, vext_nxt,
                    bqk_sb, bvb_sb, psproj))

            def fill(budget):
                while fillers and budget > 0:
                    cyc, fn = fillers.popleft()
                    fn()
                    budget -= cyc
                    while fillers and fillers[0][0] == 0:
                        _, fn2 = fillers.popleft()
                        fn2()

            # ---- attention for (r, hp) ----
            for (c0, cw) in CHUNKS:
                nsub = (cw + 127) // 128
                ssz = [min(128, cw - s * 128) for s in range(nsub)]
                out_t = out_pool.tile([128, 4, 128], F32, tag="out",
                                      name="out_t")
                # fresh psum tile per chunk (pool rotation gives clean WAR
                # edges); claim + zero each bank with one K=1 zero matmul so
                # the per-(hi,s) groups can accumulate without their own
                # start (one lazy-zero claim per 2KB bank is the hw rule)
                pctx = pctx_pool.tile([128, 2, 4, 128], F32, tag="pctx",
                                      name="pctx")

                def emit_claims():
                    # zero-write claim of both ctx banks; emitted at kt==1 (a
                    # couple of slots before the first lagged ctx accumulate)
                    # so the wait on the previous chunk's finalize copy never
                    # blocks the PE queue
                    for hi in range(2):
                        claim = nc.tensor.matmul(
                            pctx[:, hi, :, :].rearrange("p a b -> p (a b)"),
                            zeros_sb[0:1, 0:128], zeros_sb[0:1, 0:512],
                            start=True, stop=False)
                        if prev_cst[0] is not None:
                            # explicit WAR: psum reads by DVE aren't dep-
                            # tracked, order the re-claim behind the copy
                            _add_dep_helper(claim.ins, prev_cst[0].ins,
                                            sync=True,
                                            reason="pctx WAR claim-after-copy")
                ets = {}

                def emit_ctx(kt):
                    # at the last kt, emit s=0 last and put the stop there:
                    # the stop clears group state for its own out partitions
                    # only, and s=0 always has the full 128 rows the claim
                    # marked (the tail subtile can be just 90).
                    ksz = TSZ[kt]
                    et = ets.pop(kt)
                    last = kt == NT - 1
                    order = list(range(1, nsub)) + [0] if last else range(nsub)
                    for hi in range(2):
                        for s in order:
                            nc.tensor.matmul(
                                pctx[:ssz[s], hi, s, 0:65],
                                et[:ksz, hi, s * 128:s * 128 + ssz[s]],
                                vext_cur[:ksz, kt, hi, :],
                                start=False,
                                stop=(last and s == 0))

                for kt in range(NT):
                    ksz = TSZ[kt]
                    ps_s = pss.tile([128, 2, 512], F32, tag="pss", name="pss")
                    for hi in range(2):
                        p0 = hi * 64
                        nc.tensor.matmul(
                            ps_s[:ksz, hi, :cw],
                            qkT_cur[p0:p0 + 64, 1, kt * 128:kt * 128 + ksz],
                            qkT_cur[p0:p0 + 64, 0, c0:c0 + cw],
                            start=True, stop=True)
                    et = et_pool.tile([128, 2, 512], BF, tag="et", name="et")
                    nc.scalar.activation(
                        et[:ksz, :, :cw], ps_s[:ksz, :, :cw], EXP, scale=0.125)
                    ets[kt] = et
                    fill(FILLER_CYC)
                    if kt >= CTX_LAG:
                        emit_ctx(kt - CTX_LAG)
                for kt in range(NT - CTX_LAG, NT):
                    emit_ctx(kt)

                # finalize: copy to SBUF, then out = ctx * (1/denom)
                cst = cs_pool.tile([128, 2, 4, 65], F32, tag="cs", name="cst")
                prev_cst[0] = nc.vector.tensor_copy(cst[:, :, :nsub, :],
                                                    pctx[:, :, :nsub, 0:65])
                rec = rec_pool.tile([128, 2, 4], F32, tag="rec", name="rec")
                for hi in range(2):
                    nc.vector.reciprocal(
                        rec[:, hi, :nsub], cst[:, hi, :nsub, 64])
                for hi in range(2):
                    for s in range(nsub):
                        nc.vector.tensor_scalar_mul(
                            out_t[:ssz[s], s, hi * 64:(hi + 1) * 64],
                            cst[:ssz[s], hi, s, 0:64],
                            rec[:ssz[s], hi, s:s + 1])
                for s in range(nsub):
                    nc.sync.dma_start(
                        out[c0 + s * 128:c0 + s * 128 + ssz[s],
                            hp * 128:(hp + 1) * 128],
                        out_t[:ssz[s], s, :])

            # drain any leftover fillers before switching to the next hp
            while fillers:
                _, fn = fillers.popleft()
                fn()
            if si + 1 < len(steps):
                x_cur, w_cur = x_nxt, w_nxt
                qkT_cur, vext_cur = qkT_nxt, vext_nxt


def build_program(reps=1):
    nc = bacc.Bacc("TRN2", target_bir_lowering=False, debug=False,
                   num_devices=8)
    xT = nc.dram_tensor("xT", [D, S], BF, kind="ExternalInput").ap()
    wT = nc.dram_tensor("wT", [D, NHP * 384], BF, kind="ExternalInput").ap()
    bqk = nc.dram_tensor("bqk", [128, 16], F32, kind="ExternalInput").ap()
    bvb = nc.dram_tensor("bvb", [128, D], F32, kind="ExternalInput").ap()
    out = nc.dram_tensor("out", [S, D], F32, kind="ExternalOutput").ap()
    with tile.TileContext(nc) as tc:
        _body(tc, xT, wT, bqk, bvb, out, reps=reps)
    nc.compile()
    return nc


_PROGRAM = None


def _get_program():
    global _PROGRAM
    if _PROGRAM is None:
        _PROGRAM = build_program()
    return _PROGRAM


def _prep_inputs(hidden_states, Wq, bq, Wk, bk, Wv, bv):
    import ml_dtypes
    bf16 = ml_dtypes.bfloat16
    hs = np.asarray(hidden_states, dtype=np.float32)
    B = hs.shape[0]
    xT = np.ascontiguousarray(hs.transpose(0, 2, 1)).astype(bf16)
    # wT[d, hp*384 + {0:128 q, 128:256 k, 256:384 v}] so one DMA per (hp, d)
    # covers all three projections' o-tile slices.
    wq = np.asarray(Wq, dtype=np.float32).T.reshape(D, NHP, 128)
    wk = np.asarray(Wk, dtype=np.float32).T.reshape(D, NHP, 128)
    wv = np.asarray(Wv, dtype=np.float32).T.reshape(D, NHP, 128)
    wT = np.concatenate([wq, wk, wv], axis=2).reshape(D, NHP * 384)
    wT = np.ascontiguousarray(wT).astype(bf16)
    bqk_np = np.concatenate([
        np.asarray(bq, dtype=np.float32).reshape(8, 128).T,
        np.asarray(bk, dtype=np.float32).reshape(8, 128).T], axis=1)
    bqk_np = np.ascontiguousarray(bqk_np)
    bvb_np = np.ascontiguousarray(
        np.broadcast_to(np.asarray(bv, dtype=np.float32), (128, D)))
    return [{"xT": xT[b], "wT": wT, "bqk": bqk_np, "bvb": bvb_np}
            for b in range(B)]


def run(in_maps, **kw):
    nc = _get_program()
    return bass_utils.run_bass_kernel_spmd(
        nc, in_maps, core_ids=list(range(len(in_maps))), **kw)


def kernel(hidden_states, Wq, bq, Wk, bk, Wv, bv):
    in_maps = _prep_inputs(hidden_states, Wq, bq, Wk, bk, Wv, bv)
    res = run(in_maps)
    return np.stack([res.results[b]["out"] for b in range(len(in_maps))],
                    axis=0)
